# revision 6
# baseline (speedup 1.0000x reference)
"""Multi-branch BatchNorm2d (16 branches sharing one batch-stat reduction).

Computation (reference):
    mean/var over (B,H,W) per channel of x[32,64,32,32], then for each of
    N=16 branches: out[:, n*64:(n+1)*64] = gamma[n,c]*xhat + beta[n,c],
    giving out[32, 1024, 32, 32].

Strategy (8 NeuronCores, CHANNEL-parallel, no collectives, no replication):
  BatchNorm statistics are per-channel, so sharding on C (8 channels per
  core) makes both the reduction and the output fully local:
  - Core i reads only x[:, 8i:8i+8]  (1 MiB instead of the full 8 MiB a
    branch-parallel split replicates), computes mean/var for its 8
    channels over (B,H,W) with no cross-core dependency.
  - Core i writes out[:, n*64 + 8i : n*64 + 8i + 8] for all 16 branches:
    16 x 1 MiB stores = the same irreducible 16 MiB/core of output.
  HBM traffic per core: 17 MiB vs 24 MiB branch-parallel -> ~30% less.

  SBUF layout [128, 2, 1024]: partition p = c_local*16 + (b%16), free
  (b//16, h*w); every DMA line is 4 KiB contiguous. Stats pipeline: DVE
  accumulates S/NTOT (tensor_scalar accum), ACT accumulates E[x^2]
  (Square accum) per load chunk; the 16 partitions of one channel are
  then folded with 4 xor-shuffle+add rounds (stream_shuffle is a 32-way
  partition permute; each channel occupies 16 consecutive partitions).
  mean/inv fold with gamma/beta into A = gamma*inv, Bc = beta - mean*A,
  held per-partition so no broadcast is needed.
"""

import numpy as np

import concourse.bacc as bacc
import concourse.bass as bass
import concourse.tile as tile
from concourse import mybir
from concourse.bass_utils import run_bass_kernel_spmd

B, C, H, W = 32, 64, 32, 32
N = 16
NCORES = 8
CL = C // NCORES           # 8 channels per core
HW = H * W                 # 1024
BL = 16                    # batches on partitions (p = c*16 + b_lo)
BH = B // BL               # 2 free-dim batch groups
NTOT = float(B * H * W)    # 32768 elements reduced per channel
EPS = 1e-5
F32 = mybir.dt.float32

# x load chunks along the free dim (elements of the [BH, HW] = 2048-elem
# free space). Chunked so the stats pipeline drains right behind the DMA.
LCHUNK = 512               # elems per load chunk (2 KiB lines)
NLC = (BH * HW) // LCHUNK  # 4 chunks

_NC_CACHE = {}


def _build():
    nc = bacc.Bacc("TRN2", num_devices=NCORES, target_bir_lowering=False,
                   debug=False)
    # Host pre-packs x into SBUF layout: [128, 2048] with partition
    # p = c_local*16 + (b%16), free (b//16, h*w). Output is stored in the
    # same partition order ([N, 128, 2048]) and un-permuted on the host
    # during the gather; every DMA line is 8 KiB contiguous.
    x = nc.dram_tensor("x", [128, BH * HW], F32, kind="ExternalInput")
    gn = nc.dram_tensor("gn", [128, N], F32, kind="ExternalInput")
    bn = nc.dram_tensor("bn", [128, N], F32, kind="ExternalInput")
    out = nc.dram_tensor("out", [N, 128, BH * HW], F32,
                         kind="ExternalOutput")
    xr_flat = x.ap()
    out_re = out.ap()

    with tile.TileContext(nc) as tc:
        with (
            tc.tile_pool(name="xin", bufs=1) as xin,
            tc.tile_pool(name="consts", bufs=1) as consts,
            tc.tile_pool(name="small", bufs=1) as small,
            tc.tile_pool(name="outs", bufs=16) as outs,
        ):
            sbuf_eps = small.tile([128, 1], F32)
            nc.vector.memset(sbuf_eps, EPS)

            # gamma/beta pre-arranged on host: [128, 16] = [(c b_lo), n].
            g_sb = consts.tile([128, N], F32)
            b_sb = consts.tile([128, N], F32)
            nc.gpsimd.dma_start(out=g_sb, in_=gn.ap())
            nc.gpsimd.dma_start(out=b_sb, in_=bn.ap())

            # x slice load, chunked along the flattened free dim; per chunk
            # DVE accumulates the partial sum (x * 1/NTOT) and ACT the
            # partial E[x^2] (Square of x/sqrt(NTOT)) in parallel.
            x_sb = xin.tile([128, BH * HW], F32)
            x_flat = x_sb
            junk_s = small.tile([128, LCHUNK], F32, tag="junk_s")
            junk_q = small.tile([128, LCHUNK], F32, tag="junk_q")
            sq_cols = small.tile([128, 2, NLC], F32)
            for ci in range(NLC):
                f0 = ci * LCHUNK
                nc.sync.dma_start(out=x_flat[:, f0:f0 + LCHUNK],
                                  in_=xr_flat[:, f0:f0 + LCHUNK])
                nc.vector.tensor_scalar(
                    out=junk_s, in0=x_flat[:, f0:f0 + LCHUNK],
                    scalar1=1.0 / NTOT, scalar2=0.0,
                    op0=mybir.AluOpType.mult, op1=mybir.AluOpType.add,
                    accum_out=sq_cols[:, 0, ci:ci + 1].rearrange(
                        "p a -> p (a)"))
                nc.scalar.activation(
                    out=junk_q, in_=x_flat[:, f0:f0 + LCHUNK],
                    func=mybir.ActivationFunctionType.Square,
                    scale=float(NTOT ** -0.5),
                    accum_out=sq_cols[:, 1, ci:ci + 1].rearrange(
                        "p a -> p (a)"))

            # Per-partition (S, Q), then fold the 16 partitions of each
            # channel with 4 xor-rounds of the DVE 32-way partition permute.
            part = small.tile([128, 2], F32)
            nc.vector.reduce_sum(out=part, in_=sq_cols,
                                 axis=mybir.AxisListType.X)
            for k in (8, 4, 2, 1):
                shuf = small.tile([128, 2], F32, tag=f"shuf{k}")
                nc.vector.stream_shuffle(out=shuf, in_=part[:, :],
                                         mask=[i ^ k for i in range(32)])
                nxt = small.tile([128, 2], F32, tag=f"acc{k}")
                nc.vector.tensor_add(out=nxt, in0=part[:, :], in1=shuf)
                part = nxt

            # part = (mean, E[x^2]) replicated across each channel's 16
            # partitions; var = E[x^2] - mean^2 via the negated mean.
            mean = part[:, 0:1]
            nmean = small.tile([128, 1], F32)
            nc.vector.tensor_scalar_mul(out=nmean, in0=mean, scalar1=-1.0)
            var = small.tile([128, 1], F32)
            nc.vector.scalar_tensor_tensor(
                out=var, in0=nmean, scalar=mean, in1=part[:, 1:2],
                op0=mybir.AluOpType.mult, op1=mybir.AluOpType.add)
            sd = small.tile([128, 1], F32)
            nc.scalar.activation(out=sd, in_=var,
                                 func=mybir.ActivationFunctionType.Sqrt,
                                 bias=sbuf_eps[:, :])
            inv = small.tile([128, 1], F32)
            nc.vector.reciprocal(out=inv, in_=sd)

            # A = gamma*inv ; Bc = beta + nmean*A  (per (partition, branch)).
            a_sb = consts.tile([128, N], F32)
            nc.vector.tensor_scalar_mul(out=a_sb, in0=g_sb, scalar1=inv)
            bc_sb = consts.tile([128, N], F32)
            nc.vector.scalar_tensor_tensor(
                out=bc_sb, in0=a_sb, scalar=nmean, in1=b_sb,
                op0=mybir.AluOpType.mult, op1=mybir.AluOpType.add)

            # Main loop: per branch, fused out = x*A + Bc then a 1 MiB
            # store (128 partitions x 2 lines of 4 KiB). 16 distinct
            # buffers -> no reuse stalls; DVE stays ahead of the store
            # stream. The first branch is split so its store issues early.
            def fma_store(j, f0, f1):
                o = outs.tile([128, BH * HW], F32, tag="o")
                nc.vector.tensor_scalar(
                    out=o[:, f0:f1], in0=x_flat[:, f0:f1],
                    scalar1=a_sb[:, j:j + 1], scalar2=bc_sb[:, j:j + 1],
                    op0=mybir.AluOpType.mult, op1=mybir.AluOpType.add,
                )
                nc.sync.dma_start(out=out_re[j][:, f0:f1], in_=o[:, f0:f1])

            for j in range(N):
                fma_store(j, 0, BH * HW)
    nc.finalize()
    return nc


def _get_nc():
    if "nc" not in _NC_CACHE:
        _NC_CACHE["nc"] = _build()
    return _NC_CACHE["nc"]


def _run(inputs, **kwargs):
    x = np.ascontiguousarray(np.asarray(inputs["x"], dtype=np.float32))
    gamma = np.asarray(inputs["gamma"], dtype=np.float32)  # [N, C]
    beta = np.asarray(inputs["beta"], dtype=np.float32)
    # [bh, bl, cores, c, hw] so each core's packed [128, 2048] (partition
    # (c bl), free (bh hw)) is one transpose away.
    xp = x.reshape(BH, BL, NCORES, CL, HW).transpose(2, 3, 1, 0, 4)
    in_maps = []
    for i in range(NCORES):
        c0 = i * CL
        # [128, 16]: row p = c_local*16 + b_lo -> gamma[n, c0 + c_local]
        g128 = np.ascontiguousarray(
            np.repeat(gamma[:, c0:c0 + CL].T, BL, axis=0))
        b128 = np.ascontiguousarray(
            np.repeat(beta[:, c0:c0 + CL].T, BL, axis=0))
        in_maps.append({
            "x": np.ascontiguousarray(xp[i]).reshape(128, BH * HW),
            "gn": g128,
            "bn": b128,
        })
    nc = _get_nc()
    res = run_bass_kernel_spmd(nc, in_maps, core_ids=list(range(NCORES)),
                               **kwargs)
    # Core i wrote out[n, c*16+bl, bh*1024+hw] = full[bh*16+bl,
    # n*64 + i*8 + c, hw]; un-permute while gathering.
    full = np.empty((B, N * C, H, W), dtype=np.float32)
    fv = full.reshape(BH, BL, N, NCORES, CL, HW)
    for i in range(NCORES):
        arr = res.results[i]["out"].reshape(N, CL, BL, BH, HW)
        fv[:, :, :, i] = arr.transpose(3, 2, 0, 1, 4)
    return full, res


def kernel(**inputs):
    full, _ = _run(inputs)
    return full
